# revision 27
# baseline (speedup 1.0000x reference)
"""Trainium2 Bass kernel for masked causal multi-head self-attention.

Problem shapes (hardcoded): B=2, T=2048, D=1024, H=16, DH=64.

Sharding: 8 cores, tensor-parallel over (batch, head-group):
core c -> batch b = c // 4, head group g = c % 4 (heads 4g..4g+3,
feature slice 256g..256g+256). Each core computes a partial [D, T]
(transposed) output for its batch; the host sums the 4 partials per
batch and transposes back.

v4: bf16 matmuls; the PE instruction stream is kept dense by
interleaving projection chains and the previous tile's output
projection into the (exp-gated) attention inner loop, so the tensor
engine stays at full clock (HAM K=8/8). Additionally the key/value
side is COMPACTED: data_mask invalidates ~half the keys, so the host
gathers only the valid key columns (padded to a multiple of 128) and
the kernel runs K/V projections, scores, exp and AV only over
compacted keys — the causal mask becomes host-built boundary masks
for the last few k tiles of each q tile. The compacted program is
specialized to the mask's tile geometry and cached per geometry;
masks whose geometry cannot compact safely fall back to the dense
program (geom=None), so results are correct for any input.

Device algorithm per core (all matmuls bf16 in / f32 PSUM accumulate):
  scores^T = K^T x Q^T per head (two 64-partition matmuls share the
  PE via row groups), exp on ScalarE (scale=1/8, no max subtraction;
  |scores| <~ 8.2), causal/boundary masks multiplied into exp'd
  weights on DVE, AV accumulates o'^T [65, 2, 512] per head pair with
  a ones column carrying the softmax denominator; normalization
  r = dm_q/(sums+eps) via DVE recip + GPSIMD partition broadcast;
  out^T partial = Wp_c^T @ o in bf16 (host sums partials in f32).
"""

import numpy as np

B, T, D, H = 2, 2048, 1024, 16
DH = D // H          # 64
HPC = 4              # heads per core
DC = HPC * DH        # 256 feature slice per core
NC = 8               # cores
QT = 512             # q tile width
KT = 128             # k tile width (partition dim)
NQT = T // QT        # 4
NKT = T // KT        # 16
GB = 3               # boundary-mask tiles per q tile (compacted path)
SCALE = float(DH) ** -0.5

_cached = {}


MM_DTYPE = "bfloat16"  # "float32r" (accurate) or "bfloat16" (fast)


def _build_program(mm_dtype=None, geom=None):
    import concourse.tile as tile
    from concourse import bacc, mybir

    F32 = mybir.dt.float32
    MDT = getattr(mybir.dt, mm_dtype or MM_DTYPE)
    ODT = F32 if MDT == mybir.dt.float32r else MDT
    EXP = mybir.ActivationFunctionType.Exp

    if geom is None:
        TC = T                      # dense: k extent = T, causal tri masks
        nj = tuple(4 * j + 4 for j in range(NQT))
        ms = (0, 0, 0, 0)
        GBD = 1
        w0s = tuple(tuple(0 for _ in range(nj[j])) for j in range(NQT))
    else:                           # compacted keys + boundary-mask geometry
        TC, nj, ms, GBD, w0s = geom
        # column-trimming boundary tiles measured slower (shorter PE
        # bursts aggravate the HAM clock gate) - run them full width
        w0s = tuple(tuple(0 for _ in range(nj[j])) for j in range(NQT))
    NKC = TC // KT                  # k tiles
    NKB = (NKC + 3) // 4            # kT blocks of up to 512

    nc = bacc.Bacc("TRN2", target_bir_lowering=False, debug=False)

    if geom is None:
        xT_d = nc.dram_tensor("xT", [D, T], MDT, kind="ExternalInput")
    else:
        xt4_d = nc.dram_tensor("xt4", [NQT, D, QT], MDT, kind="ExternalInput")
        xk_d = nc.dram_tensor("xk", [D, TC], MDT, kind="ExternalInput")
    wq_d = nc.dram_tensor("wq", [128, D // 128, DC], MDT, kind="ExternalInput")
    wk_d = nc.dram_tensor("wk", [128, D // 128, DC], MDT, kind="ExternalInput")
    wv_d = nc.dram_tensor("wv", [128, D // 128, DC], MDT, kind="ExternalInput")
    wp_d = nc.dram_tensor("wp", [128, DC // 128, D], MDT, kind="ExternalInput")
    dm01_d = nc.dram_tensor("dm01", [KT, NKC], F32, kind="ExternalInput")
    dmr_d = nc.dram_tensor("dmrow2", [1, NQT, 2, QT], F32, kind="ExternalInput")
    if geom is None:
        tri_d = nc.dram_tensor("tri", [KT, KT], MDT, kind="ExternalInput")
    else:
        bm_d = nc.dram_tensor("bm", [KT, NQT, GBD, QT], MDT, kind="ExternalInput")
        idm_d = nc.dram_tensor("idm", [KT, KT], MDT, kind="ExternalInput")
    out_d = nc.dram_tensor("outT", [D, T], ODT, kind="ExternalOutput")

    with tile.TileContext(nc) as tc:
        with (
            tc.tile_pool(name="w", bufs=1) as wpool,
            tc.tile_pool(name="acts", bufs=1) as acts,
            tc.tile_pool(name="wt", bufs=4) as wtp,
            tc.tile_pool(name="sm", bufs=2) as sm,
            tc.tile_pool(name="ob", bufs=2) as obp,
            tc.tile_pool(name="psA", bufs=2, space="PSUM") as psA,
            tc.tile_pool(name="psS", bufs=2, space="PSUM") as psS,
            tc.tile_pool(name="psO", bufs=1, space="PSUM") as psO,
        ):
            # ---- loads (ordered by first use) ----
            wq = wpool.tile([128, D // 128, DC], MDT)
            nc.sync.dma_start(out=wq[:], in_=wq_d[:])
            xT4t = []
            if geom is None:
                xTs = []
                for kt in range(D // 128):
                    c = wpool.tile([128, T], MDT, tag=f"xt{kt}")
                    nc.sync.dma_start(
                        out=c[:], in_=xT_d[128 * kt:128 * kt + 128, :])
                    xTs.append(c)
            else:
                # only q block 0 upfront; blocks 1..3 stream in later
                c = wpool.tile([128, D // 128, QT], MDT, tag="xt4_0")
                for kt in range(D // 128):
                    nc.sync.dma_start(
                        out=c[:, kt, :],
                        in_=xt4_d[0, 128 * kt:128 * kt + 128, :])
                xT4t.append(c)
            wk = wpool.tile([128, D // 128, DC], MDT)
            nc.sync.dma_start(out=wk[:], in_=wk_d[:])
            if geom is None:
                xks = xTs
            else:
                xks = []
                for kt in range(D // 128):
                    c = wpool.tile([128, TC], MDT, tag=f"xk{kt}")
                    nc.sync.dma_start(
                        out=c[:], in_=xk_d[128 * kt:128 * kt + 128, :])
                    xks.append(c)
            wv = wpool.tile([128, D // 128, DC], MDT)
            nc.sync.dma_start(out=wv[:], in_=wv_d[:])
            dm01 = wpool.tile([KT, NKC], F32)
            nc.sync.dma_start(out=dm01[:], in_=dm01_d[:])
            if geom is None:
                tri = wpool.tile([KT, KT], MDT)
                nc.sync.dma_start(out=tri[:], in_=tri_d[:])
            else:
                bm = wpool.tile([KT, NQT, GBD, QT], MDT)
                nc.sync.dma_start(out=bm[:], in_=bm_d[:])
                idm = wpool.tile([KT, KT], MDT)
                nc.sync.dma_start(out=idm[:], in_=idm_d[:])
                for n in range(1, NQT):
                    c = wpool.tile([128, D // 128, QT], MDT, tag=f"xt4_{n}")
                    for kt in range(D // 128):
                        nc.sync.dma_start(
                            out=c[:, kt, :],
                            in_=xt4_d[n, 128 * kt:128 * kt + 128, :])
                    xT4t.append(c)
            dmr = wpool.tile([1, NQT, 2, QT], F32)
            nc.sync.dma_start(out=dmr[:], in_=dmr_d[:])
            wp = wpool.tile([128, DC // 128, D], MDT)
            nc.sync.dma_start(out=wp[:], in_=wp_d[:])

            qTn, kTn = [], []
            for n in range(NQT):
                tq = acts.tile([128, 2, QT], MDT, tag=f"qt{n}")
                qTn.append(tq)
            for n in range(NKB):
                tk = acts.tile([128, 2, min(QT, TC - QT * n)], MDT,
                               tag=f"kt{n}")
                kTn.append(tk)
            vpt = acts.tile([128, NKC, HPC, DH + 1], MDT, tag="vp")
            # ones columns (col DH) for all (t, h) in one op: vones[k] =
            # data_mask[k] rides the AV matmul as the softmax denominator
            nc.vector.tensor_scalar_mul(
                vpt[:, :, :, DH],
                dm01[:].broadcast_to([KT, NKC, HPC]),
                1.0,
            )

            def q_chain(n, m, evac):
                ps = psA.tile([128, QT], F32, tag="pa", name="ps_q")
                for kt in range(D // 128):
                    rhs = (xTs[kt][:, QT * n:QT * n + QT] if geom is None
                           else xT4t[n][:, kt, :])
                    nc.tensor.matmul(
                        ps[:],
                        wq[:, kt, 128 * m:128 * m + 128],
                        rhs,
                        start=(kt == 0), stop=(kt == D // 128 - 1),
                    )
                if evac == "scalar":
                    nc.scalar.copy(qTn[n][:, m, :], ps[:])
                else:
                    nc.vector.tensor_copy(qTn[n][:, m, :], ps[:])

            def k_chain(bk, m, evac):
                wk_ = min(QT, TC - QT * bk)
                ps = psA.tile([128, QT], F32, tag="pa", name="ps_k")
                for kt in range(D // 128):
                    nc.tensor.matmul(
                        ps[:, 0:wk_],
                        wk[:, kt, 128 * m:128 * m + 128],
                        xks[kt][:, QT * bk:QT * bk + wk_],
                        start=(kt == 0), stop=(kt == D // 128 - 1),
                    )
                if evac == "scalar":
                    nc.scalar.copy(kTn[bk][:, m, :], ps[:, 0:wk_])
                else:
                    nc.vector.tensor_copy(kTn[bk][:, m, :], ps[:, 0:wk_])

            def v_chain(t):
                ps = psA.tile([128, DC], F32, tag="pa", name="ps_v")
                for kt in range(D // 128):
                    nc.tensor.matmul(
                        ps[:],
                        xks[kt][:, 128 * t:128 * t + 128],
                        wv[:, kt, :],
                        start=(kt == 0), stop=(kt == D // 128 - 1),
                    )
                nc.vector.tensor_scalar_mul(
                    vpt[:, t, :, 0:DH],
                    ps[:].rearrange("p (h d) -> p h d", h=HPC),
                    dm01[:, t:t + 1],
                )

            def op_unit(j, dt, o_all):
                pp = psA.tile([128, QT], F32, tag="pa", name="pp")
                for kt in range(2):
                    nc.tensor.matmul(
                        pp[:],
                        wp[:, kt, 128 * dt:128 * dt + 128],
                        o_all[:, kt, :],
                        start=(kt == 0), stop=(kt == 1),
                    )
                ob = obp.tile([128, QT], ODT, tag="ob")
                if dt % 2 == 0:
                    nc.vector.tensor_copy(ob[:], pp[:])
                else:
                    nc.scalar.copy(ob[:], pp[:])
                nc.sync.dma_start(
                    out=out_d[128 * dt:128 * dt + 128, QT * j:QT * j + QT],
                    in_=ob[:],
                )

            def need_k(j):
                return (nj[j] + 3) // 4

            # ---- upfront: Q(0), K blocks for tile 0, V k-tiles < nj[0] ----
            for m in range(2):
                q_chain(0, m, "scalar")
            for bk in range(need_k(0)):
                for m in range(2):
                    k_chain(bk, m, "scalar")
            for t in range(nj[0]):
                v_chain(t)

            o_alls = []
            for j in range(NQT):
                # filler units: emitted between attention iterations to
                # keep the (in-order) PE stream dense while ScalarE exps.
                units = []
                if j + 1 < NQT:
                    for t in range(nj[j], nj[j + 1]):
                        units.append((v_chain, (t,)))
                    for m in range(2):
                        units.append((q_chain, (j + 1, m,
                                                "vector" if m else "scalar")))
                    for bk in range(need_k(j), need_k(j + 1)):
                        for m in range(2):
                            units.append((k_chain, (bk, m,
                                                    "vector" if m else "scalar")))
                reserved = []
                if j > 0:
                    for dt in range(D // 128 - 4):
                        units.append((op_unit, (j - 1, dt, o_alls[j - 1])))
                    for dt in range(D // 128 - 4, D // 128):
                        reserved.append((op_unit, (j - 1, dt, o_alls[j - 1])))
                ni = 2 * nj[j]
                emit_at = {}
                for u in range(len(units)):
                    emit_at.setdefault(u * ni // len(units), []).append(units[u])

                o_all = sm.tile([128, 2, QT], MDT, tag="oall")
                o_alls.append(o_all)
                mask_start = ms[j]
                for m in range(2):  # head pairs (2m, 2m+1)
                    o_ps = psO.tile([DH + 1, 2, QT], F32, tag="ops")
                    for i in range(nj[j]):
                        if geom is None:
                            r = i - 4 * j
                            w0 = 128 * r if r > 0 else 0
                        else:
                            r = -1
                            w0 = w0s[j][i]
                        boundary = geom is not None and i >= mask_start
                        ps_s = psS.tile([128, 2, QT], F32, tag="ps")
                        for u in range(2):
                            p0 = 64 * u
                            nc.tensor.matmul(
                                ps_s[:, u, w0:QT],
                                kTn[i // 4][p0:p0 + 64, m,
                                            128 * (i % 4):128 * (i % 4) + 128],
                                qTn[j][p0:p0 + 64, m, w0:QT],
                                start=True, stop=not boundary,
                            )
                        if boundary:
                            # causal/pad mask: accumulate -240 into masked
                            # score elements (exp then yields ~1e-13)
                            for u in range(2):
                                nc.tensor.matmul(
                                    ps_s[:, u, w0:QT],
                                    idm[:],
                                    bm[:, j, i - mask_start, w0:QT],
                                    start=False, stop=True,
                                )
                        wt = wtp.tile([128, 2, QT], MDT, tag="wt")
                        nc.scalar.activation(
                            wt[:, :, w0:QT], ps_s[:, :, w0:QT], EXP,
                            bias=0.0, scale=SCALE)
                        for u in range(2):
                            if geom is None and r >= 0:
                                # causal 128x128 diagonal block
                                nc.vector.tensor_mul(
                                    wt[:, u, w0:w0 + KT],
                                    wt[:, u, w0:w0 + KT],
                                    tri[:],
                                )
                            nc.tensor.matmul(
                                o_ps[:, u, w0:QT],
                                vpt[:, i, 2 * m + u, :],
                                wt[:, u, w0:QT],
                                start=(i == 0), stop=(i == nj[j] - 1),
                            )
                        for fn, args in emit_at.get(m * nj[j] + i, ()):
                            fn(*args)
                    # fast PSUM evacuation, then normalization off the
                    # critical path: r = dm_q / (sums + eps)
                    o_sb = sm.tile([DH + 1, 2, QT], MDT, tag="osb")
                    nc.vector.tensor_copy(o_sb[:], o_ps[:])
                    r0 = sm.tile([1, 2, QT], F32, tag="r0")
                    nc.vector.tensor_scalar_add(
                        r0[:], o_sb[DH:DH + 1, :, :], 1e-30)
                    rf = sm.tile([1, 2, QT], F32, tag="rf")
                    nc.vector.reciprocal_approx_fast(out=rf[:], in_=r0[:])
                    r2 = sm.tile([1, 2, QT], MDT, tag="r2")
                    nc.vector.tensor_mul(r2[:], rf[:], dmr[:, j, :, :])
                    rb = sm.tile([64, 2, QT], MDT, tag="rb")
                    nc.gpsimd.partition_broadcast(rb[:], r2[:], channels=64)
                    for u in range(2):
                        nc.vector.tensor_mul(
                            o_all[64 * u:64 * u + 64, m, :],
                            o_sb[0:DH, u, :], rb[:, u, :],
                        )
                for fn, args in reserved:
                    fn(*args)

            # ---- output projection for the last q tile ----
            for dt in range(D // 128):
                op_unit(NQT - 1, dt, o_alls[NQT - 1])

    nc.finalize()
    return nc


def _geometry(data_mask):
    """Compacted-key geometry shared by all cores, or None if the mask
    cannot compact profitably (then the dense program is used).

    Returns (TC, nj, ms, GBD): compacted+padded key count, number of
    k tiles per q tile, first boundary-masked tile per q tile, and the
    boundary-mask window depth."""
    dm = np.asarray(data_mask) != 0
    valids = [np.where(dm[b])[0] for b in range(B)]
    cnt = max(len(v) for v in valids)
    if cnt == 0:
        return None
    TC = KT * int(np.ceil(cnt / KT))
    if TC >= T:
        return None
    nj, ms, w0s = [], [], []
    for j in range(NQT):
        c = max(int(np.searchsorted(v, QT * (j + 1))) for v in valids)
        nj.append(max(int(np.ceil(c / KT)), 1))
        # first tile (over any batch) containing a key beyond q-tile j's
        # first position: earlier tiles are fully valid for every q
        m = min(int(np.searchsorted(v, QT * j + 1)) // KT for v in valids)
        ms.append(min(m, nj[j]))
        # per-tile column trim: every key in tile i sits at pos >=
        # min_b pos_b[128i], so q columns below that are fully masked
        w = []
        for i in range(nj[j]):
            pmin = min((int(v[KT * i]) if KT * i < len(v) else 1 << 30)
                       for v in valids)
            w.append(int(np.clip(pmin - QT * j, 0, QT - 1)))
        w0s.append(tuple(w))
    gbd = max(max(nj[j] - ms[j] for j in range(NQT)), 1)
    if gbd > 6:
        return None
    return TC, tuple(nj), tuple(ms), gbd, tuple(w0s)


def _make_in_maps(x, data_mask, Wq, Wk, Wv, Wp, mm_dtype=None, geom=None):
    if (mm_dtype or MM_DTYPE) == "bfloat16":
        import ml_dtypes
        mdt = ml_dtypes.bfloat16
    else:
        mdt = np.float32
    x = np.ascontiguousarray(np.asarray(x, np.float32))
    dm = np.asarray(data_mask).astype(np.float32)

    def wsplit(W, sl, colslice):
        W = np.asarray(W, np.float32)
        Wc = W[:, sl] if colslice else W[sl, :]
        nb = Wc.shape[0] // 128
        return np.ascontiguousarray(
            Wc.reshape(nb, 128, Wc.shape[1]).transpose(1, 0, 2).astype(mdt))

    p = np.arange(KT)[:, None]
    q = np.arange(KT)[None, :]
    tri = (q >= p).astype(np.float32).astype(mdt)

    in_maps = []
    for c in range(NC):
        b, g = divmod(c, HPC)
        sl = slice(DC * g, DC * g + DC)
        dmb = dm[b]
        dmr = np.repeat(dmb.reshape(NQT, 1, QT), 2, axis=1)[None]
        im = {
            "wq": wsplit(Wq, sl, True),
            "wk": wsplit(Wk, sl, True),
            "wv": wsplit(Wv, sl, True),
            "wp": wsplit(Wp, sl, False),
            "dmrow2": np.ascontiguousarray(dmr),
        }
        if geom is None:
            im["xT"] = np.ascontiguousarray(x[b].T.astype(mdt))
            im["dm01"] = np.ascontiguousarray(dmb.reshape(NKT, KT).T)
            im["tri"] = tri
        else:
            TC, nj, ms, GBD, _w0s = geom
            im["xt4"] = np.ascontiguousarray(
                x[b].T.reshape(D, NQT, QT).transpose(1, 0, 2).astype(mdt))
            valid = np.where(dmb != 0)[0]
            cnt = len(valid)
            xk = np.zeros((D, TC), np.float32)
            xk[:, :cnt] = x[b].T[:, valid]
            im["xk"] = np.ascontiguousarray(xk.astype(mdt))
            dm01c = np.zeros((TC // KT, KT), np.float32)
            dm01c.reshape(-1)[:cnt] = 1.0
            im["dm01"] = np.ascontiguousarray(dm01c.T)
            pos = np.full(TC, 1 << 30, np.int64)
            pos[:cnt] = valid
            bmv = np.zeros((KT, NQT, GBD, QT), np.float32)
            for j in range(NQT):
                for gg in range(GBD):
                    i = ms[j] + gg
                    if i >= nj[j]:
                        break
                    kp = pos[KT * i:KT * i + KT]  # [128]
                    qq = QT * j + np.arange(QT)   # [512]
                    bmv[:, j, gg, :] = np.where(
                        kp[:, None] <= qq[None, :], 0.0, -240.0)
            im["bm"] = np.ascontiguousarray(bmv.astype(mdt))
            im["idm"] = np.ascontiguousarray(np.eye(KT, dtype=np.float32).astype(mdt))
        in_maps.append(im)
    return in_maps


def _postprocess(results, data_mask, bp):
    out = np.empty((B, T, D), np.float32)
    for b in range(B):
        acc = results[HPC * b]["outT"].astype(np.float32)
        for g in range(1, HPC):
            acc = acc + results[HPC * b + g]["outT"].astype(np.float32)
        out[b] = acc.T
    bp = np.asarray(bp, np.float32)
    if np.any(bp):
        # general path: device skipped bp and the final row mask folding
        # assumes bp == 0, so apply both here
        out = (out + bp) * np.asarray(data_mask, np.float32)[..., None]
    return out


def _numpy_reference(x, data_mask, Wq, bq, Wk, bk, Wv, bv, Wp, bp):
    # general fallback (only used when q/k/v biases are nonzero, which
    # does not happen for this problem's setup_inputs)
    x = np.asarray(x, np.float64)
    dm = np.asarray(data_mask) != 0
    q = (x @ np.asarray(Wq, np.float64) + np.asarray(bq, np.float64))
    k = (x @ np.asarray(Wk, np.float64) + np.asarray(bk, np.float64))
    v = (x @ np.asarray(Wv, np.float64) + np.asarray(bv, np.float64))
    q = q.reshape(B, T, H, DH).transpose(0, 2, 1, 3) * SCALE
    k = k.reshape(B, T, H, DH).transpose(0, 2, 1, 3)
    v = v.reshape(B, T, H, DH).transpose(0, 2, 1, 3)
    causal = np.tril(np.ones((T, T), bool))
    out = np.empty((B, T, D), np.float64)
    for b in range(B):
        mask = causal & dm[b][:, None] & dm[b][None, :]
        for h in range(H):
            s = q[b, h] @ k[b, h].T
            s = np.where(mask, s, -np.inf)
            s -= np.max(s, axis=-1, keepdims=True)
            w = np.exp(s)
            denom = w.sum(-1, keepdims=True)
            w = np.where(denom > 0, w / np.where(denom == 0, 1, denom), 0.0)
            w = np.nan_to_num(w)
            out[b, :, h * DH:(h + 1) * DH] = w @ v[b, h]
    out = out @ np.asarray(Wp, np.float64) + np.asarray(bp, np.float64)
    out *= dm[..., None]
    return out.astype(np.float32)


def kernel(x, data_mask, Wq, bq, Wk, bk, Wv, bv, Wp, bp):
    if any(np.any(np.asarray(v)) for v in (bq, bk, bv)):
        return _numpy_reference(x, data_mask, Wq, bq, Wk, bk, Wv, bv, Wp, bp)

    from concourse.bass_utils import run_bass_kernel_spmd

    geom = _geometry(data_mask)
    key = ("nc", geom)
    if key not in _cached:
        _cached[key] = _build_program(geom=geom)
    nc = _cached[key]
    in_maps = _make_in_maps(x, data_mask, Wq, Wk, Wv, Wp, geom=geom)
    res = run_bass_kernel_spmd(nc, in_maps, core_ids=list(range(NC)))
    return _postprocess(res.results, data_mask, bp)


# revision 28
# speedup vs baseline: 1.1115x; 1.1115x over previous
"""Trainium2 Bass kernel for masked causal multi-head self-attention.

Problem shapes (hardcoded): B=2, T=2048, D=1024, H=16, DH=64.

Sharding: 8 cores, tensor-parallel over (batch, head-group):
core c -> batch b = c // 4, head group g = c % 4 (heads 4g..4g+3,
feature slice 256g..256g+256). Each core computes a partial [D, T]
(transposed) output for its batch; the host sums the 4 partials per
batch and transposes back.

v4: bf16 matmuls; the PE instruction stream is kept dense by
interleaving projection chains and the previous tile's output
projection into the (exp-gated) attention inner loop, so the tensor
engine stays at full clock (HAM K=8/8). Additionally the key/value
side is COMPACTED: data_mask invalidates ~half the keys, so the host
gathers only the valid key columns (padded to a multiple of 128) and
the kernel runs K/V projections, scores, exp and AV only over
compacted keys — the causal mask becomes host-built boundary masks
for the last few k tiles of each q tile. The compacted program is
specialized to the mask's tile geometry and cached per geometry;
masks whose geometry cannot compact safely fall back to the dense
program (geom=None), so results are correct for any input.

Device algorithm per core (all matmuls bf16 in / f32 PSUM accumulate):
  scores^T = K^T x Q^T per head (two 64-partition matmuls share the
  PE via row groups), exp on ScalarE (scale=1/8, no max subtraction;
  |scores| <~ 8.2), causal/boundary masks multiplied into exp'd
  weights on DVE, AV accumulates o'^T [65, 2, 512] per head pair with
  a ones column carrying the softmax denominator; normalization
  r = dm_q/(sums+eps) via DVE recip + GPSIMD partition broadcast;
  out^T partial = Wp_c^T @ o in bf16 (host sums partials in f32).
"""

import numpy as np

B, T, D, H = 2, 2048, 1024, 16
DH = D // H          # 64
HPC = 4              # heads per core
DC = HPC * DH        # 256 feature slice per core
NC = 8               # cores
QT = 512             # q tile width
KT = 128             # k tile width (partition dim)
NQT = T // QT        # 4
NKT = T // KT        # 16
GB = 3               # boundary-mask tiles per q tile (compacted path)
SCALE = float(DH) ** -0.5

_cached = {}


MM_DTYPE = "bfloat16"  # "float32r" (accurate) or "bfloat16" (fast)


def _build_program(mm_dtype=None, geom=None):
    import concourse.tile as tile
    from concourse import bacc, mybir

    F32 = mybir.dt.float32
    MDT = getattr(mybir.dt, mm_dtype or MM_DTYPE)
    ODT = F32 if MDT == mybir.dt.float32r else MDT
    EXP = mybir.ActivationFunctionType.Exp

    if geom is None:
        TC = T                      # dense: k extent = T, causal tri masks
        nj = tuple(4 * j + 4 for j in range(NQT))
        ms = (0, 0, 0, 0)
        GBD = 1
        w0s = tuple(tuple(0 for _ in range(nj[j])) for j in range(NQT))
    else:                           # compacted keys + boundary-mask geometry
        TC, nj, ms, GBD, w0s = geom
        # column-trimming boundary tiles measured slower (shorter PE
        # bursts aggravate the HAM clock gate) - run them full width
        w0s = tuple(tuple(0 for _ in range(nj[j])) for j in range(NQT))
    NKC = TC // KT                  # k tiles
    NKB = (NKC + 3) // 4            # kT blocks of up to 512

    nc = bacc.Bacc("TRN2", target_bir_lowering=False, debug=False)

    xT_d = nc.dram_tensor("xT", [D, T], MDT, kind="ExternalInput")
    if geom is not None:
        xk_d = nc.dram_tensor("xk", [D, TC], MDT, kind="ExternalInput")
    wq_d = nc.dram_tensor("wq", [128, D // 128, DC], MDT, kind="ExternalInput")
    wk_d = nc.dram_tensor("wk", [128, D // 128, DC], MDT, kind="ExternalInput")
    wv_d = nc.dram_tensor("wv", [128, D // 128, DC], MDT, kind="ExternalInput")
    wp_d = nc.dram_tensor("wp", [128, DC // 128, D], MDT, kind="ExternalInput")
    dm01_d = nc.dram_tensor("dm01", [KT, NKC], F32, kind="ExternalInput")
    dmr_d = nc.dram_tensor("dmrow2", [1, NQT, 2, QT], F32, kind="ExternalInput")
    if geom is None:
        tri_d = nc.dram_tensor("tri", [KT, KT], MDT, kind="ExternalInput")
    else:
        bm_d = nc.dram_tensor("bm", [KT, NQT, GBD, QT], MDT, kind="ExternalInput")
        idm_d = nc.dram_tensor("idm", [KT, KT], MDT, kind="ExternalInput")
    out_d = nc.dram_tensor("outT", [D, T], ODT, kind="ExternalOutput")

    with tile.TileContext(nc) as tc:
        with (
            tc.tile_pool(name="w", bufs=1) as wpool,
            tc.tile_pool(name="acts", bufs=1) as acts,
            tc.tile_pool(name="wt", bufs=4) as wtp,
            tc.tile_pool(name="sm", bufs=2) as sm,
            tc.tile_pool(name="ob", bufs=2) as obp,
            tc.tile_pool(name="psA", bufs=2, space="PSUM") as psA,
            tc.tile_pool(name="psS", bufs=2, space="PSUM") as psS,
            tc.tile_pool(name="psO", bufs=1, space="PSUM") as psO,
        ):
            # ---- loads (ordered by first use) ----
            wq = wpool.tile([128, D // 128, DC], MDT)
            nc.sync.dma_start(out=wq[:], in_=wq_d[:])
            xTs = []
            for kt in range(D // 128):
                c = wpool.tile([128, T], MDT, tag=f"xt{kt}")
                nc.sync.dma_start(out=c[:], in_=xT_d[128 * kt:128 * kt + 128, :])
                xTs.append(c)
            wk = wpool.tile([128, D // 128, DC], MDT)
            nc.sync.dma_start(out=wk[:], in_=wk_d[:])
            if geom is None:
                xks = xTs
            else:
                xks = []
                for kt in range(D // 128):
                    c = wpool.tile([128, TC], MDT, tag=f"xk{kt}")
                    nc.sync.dma_start(
                        out=c[:], in_=xk_d[128 * kt:128 * kt + 128, :])
                    xks.append(c)
            wv = wpool.tile([128, D // 128, DC], MDT)
            nc.sync.dma_start(out=wv[:], in_=wv_d[:])
            dm01 = wpool.tile([KT, NKC], F32)
            nc.sync.dma_start(out=dm01[:], in_=dm01_d[:])
            if geom is None:
                tri = wpool.tile([KT, KT], MDT)
                nc.sync.dma_start(out=tri[:], in_=tri_d[:])
            else:
                bm = wpool.tile([KT, NQT, GBD, QT], MDT)
                nc.sync.dma_start(out=bm[:], in_=bm_d[:])
                idm = wpool.tile([KT, KT], MDT)
                nc.sync.dma_start(out=idm[:], in_=idm_d[:])
            dmr = wpool.tile([1, NQT, 2, QT], F32)
            nc.sync.dma_start(out=dmr[:], in_=dmr_d[:])
            wp = wpool.tile([128, DC // 128, D], MDT)
            nc.sync.dma_start(out=wp[:], in_=wp_d[:])

            qTn, kTn = [], []
            for n in range(NQT):
                tq = acts.tile([128, 2, QT], MDT, tag=f"qt{n}")
                qTn.append(tq)
            for n in range(NKB):
                tk = acts.tile([128, 2, min(QT, TC - QT * n)], MDT,
                               tag=f"kt{n}")
                kTn.append(tk)
            vpt = acts.tile([128, NKC, HPC, DH + 1], MDT, tag="vp")
            # ones columns (col DH) for all (t, h) in one op: vones[k] =
            # data_mask[k] rides the AV matmul as the softmax denominator
            nc.vector.tensor_scalar_mul(
                vpt[:, :, :, DH],
                dm01[:].broadcast_to([KT, NKC, HPC]),
                1.0,
            )

            def q_chain(n, m, evac):
                ps = psA.tile([128, QT], F32, tag="pa", name="ps_q")
                for kt in range(D // 128):
                    nc.tensor.matmul(
                        ps[:],
                        wq[:, kt, 128 * m:128 * m + 128],
                        xTs[kt][:, QT * n:QT * n + QT],
                        start=(kt == 0), stop=(kt == D // 128 - 1),
                    )
                if evac == "scalar":
                    nc.scalar.copy(qTn[n][:, m, :], ps[:])
                else:
                    nc.vector.tensor_copy(qTn[n][:, m, :], ps[:])

            def k_chain(bk, m, evac):
                wk_ = min(QT, TC - QT * bk)
                ps = psA.tile([128, QT], F32, tag="pa", name="ps_k")
                for kt in range(D // 128):
                    nc.tensor.matmul(
                        ps[:, 0:wk_],
                        wk[:, kt, 128 * m:128 * m + 128],
                        xks[kt][:, QT * bk:QT * bk + wk_],
                        start=(kt == 0), stop=(kt == D // 128 - 1),
                    )
                if evac == "scalar":
                    nc.scalar.copy(kTn[bk][:, m, :], ps[:, 0:wk_])
                else:
                    nc.vector.tensor_copy(kTn[bk][:, m, :], ps[:, 0:wk_])

            def v_chain(t):
                ps = psA.tile([128, DC], F32, tag="pa", name="ps_v")
                for kt in range(D // 128):
                    nc.tensor.matmul(
                        ps[:],
                        xks[kt][:, 128 * t:128 * t + 128],
                        wv[:, kt, :],
                        start=(kt == 0), stop=(kt == D // 128 - 1),
                    )
                nc.vector.tensor_scalar_mul(
                    vpt[:, t, :, 0:DH],
                    ps[:].rearrange("p (h d) -> p h d", h=HPC),
                    dm01[:, t:t + 1],
                )

            def op_unit(j, dt, o_all):
                pp = psA.tile([128, QT], F32, tag="pa", name="pp")
                for kt in range(2):
                    nc.tensor.matmul(
                        pp[:],
                        wp[:, kt, 128 * dt:128 * dt + 128],
                        o_all[:, kt, :],
                        start=(kt == 0), stop=(kt == 1),
                    )
                ob = obp.tile([128, QT], ODT, tag="ob")
                if dt % 2 == 0:
                    nc.vector.tensor_copy(ob[:], pp[:])
                else:
                    nc.scalar.copy(ob[:], pp[:])
                nc.sync.dma_start(
                    out=out_d[128 * dt:128 * dt + 128, QT * j:QT * j + QT],
                    in_=ob[:],
                )

            def need_k(j):
                return (nj[j] + 3) // 4

            # ---- upfront: Q(0), K blocks for tile 0, V k-tiles < nj[0] ----
            for m in range(2):
                q_chain(0, m, "scalar")
            for bk in range(need_k(0)):
                for m in range(2):
                    k_chain(bk, m, "scalar")
            for t in range(nj[0]):
                v_chain(t)

            o_alls = []
            for j in range(NQT):
                # filler units: emitted between attention iterations to
                # keep the (in-order) PE stream dense while ScalarE exps.
                units = []
                if j + 1 < NQT:
                    for t in range(nj[j], nj[j + 1]):
                        units.append((v_chain, (t,)))
                    for m in range(2):
                        units.append((q_chain, (j + 1, m,
                                                "vector" if m else "scalar")))
                    for bk in range(need_k(j), need_k(j + 1)):
                        for m in range(2):
                            units.append((k_chain, (bk, m,
                                                    "vector" if m else "scalar")))
                if j > 0:
                    for dt in range(D // 128):
                        units.append((op_unit, (j - 1, dt, o_alls[j - 1])))
                ni = 2 * nj[j]
                emit_at = {}
                for u in range(len(units)):
                    emit_at.setdefault(u * ni // len(units), []).append(units[u])

                o_all = sm.tile([128, 2, QT], MDT, tag="oall")
                o_alls.append(o_all)
                mask_start = ms[j]
                for m in range(2):  # head pairs (2m, 2m+1)
                    o_ps = psO.tile([DH + 1, 2, QT], F32, tag="ops")
                    for i in range(nj[j]):
                        if geom is None:
                            r = i - 4 * j
                            w0 = 128 * r if r > 0 else 0
                        else:
                            r = -1
                            w0 = w0s[j][i]
                        boundary = geom is not None and i >= mask_start
                        ps_s = psS.tile([128, 2, QT], F32, tag="ps")
                        for u in range(2):
                            p0 = 64 * u
                            nc.tensor.matmul(
                                ps_s[:, u, w0:QT],
                                kTn[i // 4][p0:p0 + 64, m,
                                            128 * (i % 4):128 * (i % 4) + 128],
                                qTn[j][p0:p0 + 64, m, w0:QT],
                                start=True, stop=not boundary,
                            )
                        if boundary:
                            # causal/pad mask: accumulate -240 into masked
                            # score elements (exp then yields ~1e-13)
                            for u in range(2):
                                nc.tensor.matmul(
                                    ps_s[:, u, w0:QT],
                                    idm[:],
                                    bm[:, j, i - mask_start, w0:QT],
                                    start=False, stop=True,
                                )
                        wt = wtp.tile([128, 2, QT], MDT, tag="wt")
                        nc.scalar.activation(
                            wt[:, :, w0:QT], ps_s[:, :, w0:QT], EXP,
                            bias=0.0, scale=SCALE)
                        for u in range(2):
                            if geom is None and r >= 0:
                                # causal 128x128 diagonal block
                                nc.vector.tensor_mul(
                                    wt[:, u, w0:w0 + KT],
                                    wt[:, u, w0:w0 + KT],
                                    tri[:],
                                )
                            nc.tensor.matmul(
                                o_ps[:, u, w0:QT],
                                vpt[:, i, 2 * m + u, :],
                                wt[:, u, w0:QT],
                                start=(i == 0), stop=(i == nj[j] - 1),
                            )
                        for fn, args in emit_at.get(m * nj[j] + i, ()):
                            fn(*args)
                    # fast PSUM evacuation, then normalization off the
                    # critical path: r = dm_q / (sums + eps)
                    o_sb = sm.tile([DH + 1, 2, QT], MDT, tag="osb")
                    nc.vector.tensor_copy(o_sb[:], o_ps[:])
                    r0 = sm.tile([1, 2, QT], F32, tag="r0")
                    nc.vector.tensor_scalar_add(
                        r0[:], o_sb[DH:DH + 1, :, :], 1e-30)
                    rf = sm.tile([1, 2, QT], F32, tag="rf")
                    nc.vector.reciprocal_approx_fast(out=rf[:], in_=r0[:])
                    r2 = sm.tile([1, 2, QT], MDT, tag="r2")
                    nc.vector.tensor_mul(r2[:], rf[:], dmr[:, j, :, :])
                    rb = sm.tile([64, 2, QT], MDT, tag="rb")
                    nc.gpsimd.partition_broadcast(rb[:], r2[:], channels=64)
                    for u in range(2):
                        nc.vector.tensor_mul(
                            o_all[64 * u:64 * u + 64, m, :],
                            o_sb[0:DH, u, :], rb[:, u, :],
                        )

            # ---- output projection for the last q tile ----
            for dt in range(D // 128):
                op_unit(NQT - 1, dt, o_alls[NQT - 1])

    nc.finalize()
    return nc


def _geometry(data_mask):
    """Compacted-key geometry shared by all cores, or None if the mask
    cannot compact profitably (then the dense program is used).

    Returns (TC, nj, ms, GBD): compacted+padded key count, number of
    k tiles per q tile, first boundary-masked tile per q tile, and the
    boundary-mask window depth."""
    dm = np.asarray(data_mask) != 0
    valids = [np.where(dm[b])[0] for b in range(B)]
    cnt = max(len(v) for v in valids)
    if cnt == 0:
        return None
    TC = KT * int(np.ceil(cnt / KT))
    if TC >= T:
        return None
    nj, ms, w0s = [], [], []
    for j in range(NQT):
        c = max(int(np.searchsorted(v, QT * (j + 1))) for v in valids)
        nj.append(max(int(np.ceil(c / KT)), 1))
        # first tile (over any batch) containing a key beyond q-tile j's
        # first position: earlier tiles are fully valid for every q
        m = min(int(np.searchsorted(v, QT * j + 1)) // KT for v in valids)
        ms.append(min(m, nj[j]))
        # per-tile column trim: every key in tile i sits at pos >=
        # min_b pos_b[128i], so q columns below that are fully masked
        w = []
        for i in range(nj[j]):
            pmin = min((int(v[KT * i]) if KT * i < len(v) else 1 << 30)
                       for v in valids)
            w.append(int(np.clip(pmin - QT * j, 0, QT - 1)))
        w0s.append(tuple(w))
    gbd = max(max(nj[j] - ms[j] for j in range(NQT)), 1)
    if gbd > 6:
        return None
    return TC, tuple(nj), tuple(ms), gbd, tuple(w0s)


def _make_in_maps(x, data_mask, Wq, Wk, Wv, Wp, mm_dtype=None, geom=None):
    if (mm_dtype or MM_DTYPE) == "bfloat16":
        import ml_dtypes
        mdt = ml_dtypes.bfloat16
    else:
        mdt = np.float32
    x = np.ascontiguousarray(np.asarray(x, np.float32))
    dm = np.asarray(data_mask).astype(np.float32)

    def wsplit(W, sl, colslice):
        W = np.asarray(W, np.float32)
        Wc = W[:, sl] if colslice else W[sl, :]
        nb = Wc.shape[0] // 128
        return np.ascontiguousarray(
            Wc.reshape(nb, 128, Wc.shape[1]).transpose(1, 0, 2).astype(mdt))

    p = np.arange(KT)[:, None]
    q = np.arange(KT)[None, :]
    tri = (q >= p).astype(np.float32).astype(mdt)

    in_maps = []
    for c in range(NC):
        b, g = divmod(c, HPC)
        sl = slice(DC * g, DC * g + DC)
        dmb = dm[b]
        dmr = np.repeat(dmb.reshape(NQT, 1, QT), 2, axis=1)[None]
        im = {
            "xT": np.ascontiguousarray(x[b].T.astype(mdt)),
            "wq": wsplit(Wq, sl, True),
            "wk": wsplit(Wk, sl, True),
            "wv": wsplit(Wv, sl, True),
            "wp": wsplit(Wp, sl, False),
            "dmrow2": np.ascontiguousarray(dmr),
        }
        if geom is None:
            im["dm01"] = np.ascontiguousarray(dmb.reshape(NKT, KT).T)
            im["tri"] = tri
        else:
            TC, nj, ms, GBD, _w0s = geom
            valid = np.where(dmb != 0)[0]
            cnt = len(valid)
            xk = np.zeros((D, TC), np.float32)
            xk[:, :cnt] = x[b].T[:, valid]
            im["xk"] = np.ascontiguousarray(xk.astype(mdt))
            dm01c = np.zeros((TC // KT, KT), np.float32)
            dm01c.reshape(-1)[:cnt] = 1.0
            im["dm01"] = np.ascontiguousarray(dm01c.T)
            pos = np.full(TC, 1 << 30, np.int64)
            pos[:cnt] = valid
            bmv = np.zeros((KT, NQT, GBD, QT), np.float32)
            for j in range(NQT):
                for gg in range(GBD):
                    i = ms[j] + gg
                    if i >= nj[j]:
                        break
                    kp = pos[KT * i:KT * i + KT]  # [128]
                    qq = QT * j + np.arange(QT)   # [512]
                    bmv[:, j, gg, :] = np.where(
                        kp[:, None] <= qq[None, :], 0.0, -240.0)
            im["bm"] = np.ascontiguousarray(bmv.astype(mdt))
            im["idm"] = np.ascontiguousarray(np.eye(KT, dtype=np.float32).astype(mdt))
        in_maps.append(im)
    return in_maps


def _postprocess(results, data_mask, bp):
    out = np.empty((B, T, D), np.float32)
    for b in range(B):
        acc = results[HPC * b]["outT"].astype(np.float32)
        for g in range(1, HPC):
            acc = acc + results[HPC * b + g]["outT"].astype(np.float32)
        out[b] = acc.T
    bp = np.asarray(bp, np.float32)
    if np.any(bp):
        # general path: device skipped bp and the final row mask folding
        # assumes bp == 0, so apply both here
        out = (out + bp) * np.asarray(data_mask, np.float32)[..., None]
    return out


def _numpy_reference(x, data_mask, Wq, bq, Wk, bk, Wv, bv, Wp, bp):
    # general fallback (only used when q/k/v biases are nonzero, which
    # does not happen for this problem's setup_inputs)
    x = np.asarray(x, np.float64)
    dm = np.asarray(data_mask) != 0
    q = (x @ np.asarray(Wq, np.float64) + np.asarray(bq, np.float64))
    k = (x @ np.asarray(Wk, np.float64) + np.asarray(bk, np.float64))
    v = (x @ np.asarray(Wv, np.float64) + np.asarray(bv, np.float64))
    q = q.reshape(B, T, H, DH).transpose(0, 2, 1, 3) * SCALE
    k = k.reshape(B, T, H, DH).transpose(0, 2, 1, 3)
    v = v.reshape(B, T, H, DH).transpose(0, 2, 1, 3)
    causal = np.tril(np.ones((T, T), bool))
    out = np.empty((B, T, D), np.float64)
    for b in range(B):
        mask = causal & dm[b][:, None] & dm[b][None, :]
        for h in range(H):
            s = q[b, h] @ k[b, h].T
            s = np.where(mask, s, -np.inf)
            s -= np.max(s, axis=-1, keepdims=True)
            w = np.exp(s)
            denom = w.sum(-1, keepdims=True)
            w = np.where(denom > 0, w / np.where(denom == 0, 1, denom), 0.0)
            w = np.nan_to_num(w)
            out[b, :, h * DH:(h + 1) * DH] = w @ v[b, h]
    out = out @ np.asarray(Wp, np.float64) + np.asarray(bp, np.float64)
    out *= dm[..., None]
    return out.astype(np.float32)


def kernel(x, data_mask, Wq, bq, Wk, bk, Wv, bv, Wp, bp):
    if any(np.any(np.asarray(v)) for v in (bq, bk, bv)):
        return _numpy_reference(x, data_mask, Wq, bq, Wk, bk, Wv, bv, Wp, bp)

    from concourse.bass_utils import run_bass_kernel_spmd

    geom = _geometry(data_mask)
    key = ("nc", geom)
    if key not in _cached:
        _cached[key] = _build_program(geom=geom)
    nc = _cached[key]
    in_maps = _make_in_maps(x, data_mask, Wq, Wk, Wv, Wp, geom=geom)
    res = run_bass_kernel_spmd(nc, in_maps, core_ids=list(range(NC)))
    return _postprocess(res.results, data_mask, bp)


# revision 29
# speedup vs baseline: 1.1208x; 1.0084x over previous
"""Trainium2 Bass kernel for masked causal multi-head self-attention.

Problem shapes (hardcoded): B=2, T=2048, D=1024, H=16, DH=64.

Sharding: 8 cores, tensor-parallel over (batch, head-group):
core c -> batch b = c // 4, head group g = c % 4 (heads 4g..4g+3,
feature slice 256g..256g+256). Each core computes a partial [D, T]
(transposed) output for its batch; the host sums the 4 partials per
batch and transposes back.

v4: bf16 matmuls; the PE instruction stream is kept dense by
interleaving projection chains and the previous tile's output
projection into the (exp-gated) attention inner loop, so the tensor
engine stays at full clock (HAM K=8/8). Additionally the key/value
side is COMPACTED: data_mask invalidates ~half the keys, so the host
gathers only the valid key columns (padded to a multiple of 128) and
the kernel runs K/V projections, scores, exp and AV only over
compacted keys — the causal mask becomes host-built boundary masks
for the last few k tiles of each q tile. The compacted program is
specialized to the mask's tile geometry and cached per geometry;
masks whose geometry cannot compact safely fall back to the dense
program (geom=None), so results are correct for any input.

Device algorithm per core (all matmuls bf16 in / f32 PSUM accumulate):
  scores^T = K^T x Q^T per head (two 64-partition matmuls share the
  PE via row groups), exp on ScalarE (scale=1/8, no max subtraction;
  |scores| <~ 8.2), causal/boundary masks multiplied into exp'd
  weights on DVE, AV accumulates o'^T [65, 2, 512] per head pair with
  a ones column carrying the softmax denominator; normalization
  r = dm_q/(sums+eps) via DVE recip + GPSIMD partition broadcast;
  out^T partial = Wp_c^T @ o in bf16 (host sums partials in f32).
"""

import numpy as np

B, T, D, H = 2, 2048, 1024, 16
DH = D // H          # 64
HPC = 4              # heads per core
DC = HPC * DH        # 256 feature slice per core
NC = 8               # cores
QT = 512             # q tile width
KT = 128             # k tile width (partition dim)
NQT = T // QT        # 4
NKT = T // KT        # 16
GB = 3               # boundary-mask tiles per q tile (compacted path)
SCALE = float(DH) ** -0.5

_cached = {}


MM_DTYPE = "bfloat16"  # "float32r" (accurate) or "bfloat16" (fast)


def _build_program(mm_dtype=None, geom=None):
    import concourse.tile as tile
    from concourse import bacc, mybir

    F32 = mybir.dt.float32
    MDT = getattr(mybir.dt, mm_dtype or MM_DTYPE)
    ODT = F32 if MDT == mybir.dt.float32r else MDT
    EXP = mybir.ActivationFunctionType.Exp

    if geom is None:
        TC = T                      # dense: k extent = T, causal tri masks
        nj = tuple(4 * j + 4 for j in range(NQT))
        ms = (0, 0, 0, 0)
        GBD = 1
        w0s = tuple(tuple(0 for _ in range(nj[j])) for j in range(NQT))
    else:                           # compacted keys + boundary-mask geometry
        TC, nj, ms, GBD, w0s = geom
        # column-trimming boundary tiles measured slower (shorter PE
        # bursts aggravate the HAM clock gate) - run them full width
        w0s = tuple(tuple(0 for _ in range(nj[j])) for j in range(NQT))
    NKC = TC // KT                  # k tiles
    NKB = (NKC + 3) // 4            # kT blocks of up to 512

    nc = bacc.Bacc("TRN2", target_bir_lowering=False, debug=False)

    xT_d = nc.dram_tensor("xT", [D, T], MDT, kind="ExternalInput")
    if geom is not None:
        xk_d = nc.dram_tensor("xk", [D, TC], MDT, kind="ExternalInput")
    wq_d = nc.dram_tensor("wq", [128, D // 128, DC], MDT, kind="ExternalInput")
    wk_d = nc.dram_tensor("wk", [128, D // 128, DC], MDT, kind="ExternalInput")
    wv_d = nc.dram_tensor("wv", [128, D // 128, DC], MDT, kind="ExternalInput")
    wp_d = nc.dram_tensor("wp", [128, DC // 128, D], MDT, kind="ExternalInput")
    dm01_d = nc.dram_tensor("dm01", [KT, NKC], F32, kind="ExternalInput")
    dmr_d = nc.dram_tensor("dmrow2", [1, NQT, 2, QT], F32, kind="ExternalInput")
    if geom is None:
        tri_d = nc.dram_tensor("tri", [KT, KT], MDT, kind="ExternalInput")
    else:
        bm_d = nc.dram_tensor("bm", [KT, NQT, GBD, QT], MDT, kind="ExternalInput")
        idm_d = nc.dram_tensor("idm", [KT, KT], MDT, kind="ExternalInput")
    out_d = nc.dram_tensor("outT", [D, T], ODT, kind="ExternalOutput")

    with tile.TileContext(nc) as tc:
        with (
            tc.tile_pool(name="w", bufs=1) as wpool,
            tc.tile_pool(name="acts", bufs=1) as acts,
            tc.tile_pool(name="wt", bufs=4) as wtp,
            tc.tile_pool(name="sm", bufs=2) as sm,
            tc.tile_pool(name="ob", bufs=2) as obp,
            tc.tile_pool(name="psA", bufs=2, space="PSUM") as psA,
            tc.tile_pool(name="psS", bufs=2, space="PSUM") as psS,
            tc.tile_pool(name="psO", bufs=1, space="PSUM") as psO,
        ):
            # ---- loads (ordered by first use) ----
            wq = wpool.tile([128, D // 128, DC], MDT)
            nc.sync.dma_start(out=wq[:], in_=wq_d[:])
            xTs = []
            for kt in range(D // 128):
                c = wpool.tile([128, T], MDT, tag=f"xt{kt}")
                nc.sync.dma_start(out=c[:], in_=xT_d[128 * kt:128 * kt + 128, :])
                xTs.append(c)
            wk = wpool.tile([128, D // 128, DC], MDT)
            nc.sync.dma_start(out=wk[:], in_=wk_d[:])
            if geom is None:
                xks = xTs
            else:
                xks = []
                for kt in range(D // 128):
                    c = wpool.tile([128, TC], MDT, tag=f"xk{kt}")
                    nc.sync.dma_start(
                        out=c[:], in_=xk_d[128 * kt:128 * kt + 128, :])
                    xks.append(c)
            wv = wpool.tile([128, D // 128, DC], MDT)
            nc.sync.dma_start(out=wv[:], in_=wv_d[:])
            dm01 = wpool.tile([KT, NKC], F32)
            nc.sync.dma_start(out=dm01[:], in_=dm01_d[:])
            if geom is None:
                tri = wpool.tile([KT, KT], MDT)
                nc.sync.dma_start(out=tri[:], in_=tri_d[:])
            else:
                bm = wpool.tile([KT, NQT, GBD, QT], MDT)
                nc.sync.dma_start(out=bm[:], in_=bm_d[:])
                idm = wpool.tile([KT, KT], MDT)
                nc.sync.dma_start(out=idm[:], in_=idm_d[:])
            dmr = wpool.tile([1, NQT, 2, QT], F32)
            nc.sync.dma_start(out=dmr[:], in_=dmr_d[:])
            wp = wpool.tile([128, DC // 128, D], MDT)
            nc.sync.dma_start(out=wp[:], in_=wp_d[:])

            qTn, kTn = [], []
            for n in range(NQT):
                tq = acts.tile([128, 2, QT], MDT, tag=f"qt{n}")
                qTn.append(tq)
            for n in range(NKB):
                tk = acts.tile([128, 2, min(QT, TC - QT * n)], MDT,
                               tag=f"kt{n}")
                kTn.append(tk)
            vpt = acts.tile([128, NKC, HPC, DH + 1], MDT, tag="vp")
            # ones columns (col DH) for all (t, h) in one op: vones[k] =
            # data_mask[k] rides the AV matmul as the softmax denominator
            nc.vector.tensor_scalar_mul(
                vpt[:, :, :, DH],
                dm01[:].broadcast_to([KT, NKC, HPC]),
                1.0,
            )

            def q_chain(n, m, evac):
                ps = psA.tile([128, QT], F32, tag="pa", name="ps_q")
                for kt in range(D // 128):
                    nc.tensor.matmul(
                        ps[:],
                        wq[:, kt, 128 * m:128 * m + 128],
                        xTs[kt][:, QT * n:QT * n + QT],
                        start=(kt == 0), stop=(kt == D // 128 - 1),
                    )
                if evac == "scalar":
                    nc.scalar.copy(qTn[n][:, m, :], ps[:])
                else:
                    nc.vector.tensor_copy(qTn[n][:, m, :], ps[:])

            def k_chain(bk, m, evac):
                wk_ = min(QT, TC - QT * bk)
                ps = psA.tile([128, QT], F32, tag="pa", name="ps_k")
                for kt in range(D // 128):
                    nc.tensor.matmul(
                        ps[:, 0:wk_],
                        wk[:, kt, 128 * m:128 * m + 128],
                        xks[kt][:, QT * bk:QT * bk + wk_],
                        start=(kt == 0), stop=(kt == D // 128 - 1),
                    )
                if evac == "scalar":
                    nc.scalar.copy(kTn[bk][:, m, :], ps[:, 0:wk_])
                else:
                    nc.vector.tensor_copy(kTn[bk][:, m, :], ps[:, 0:wk_])

            def v_chain(t):
                ps = psA.tile([128, DC], F32, tag="pa", name="ps_v")
                for kt in range(D // 128):
                    nc.tensor.matmul(
                        ps[:],
                        xks[kt][:, 128 * t:128 * t + 128],
                        wv[:, kt, :],
                        start=(kt == 0), stop=(kt == D // 128 - 1),
                    )
                nc.vector.tensor_scalar_mul(
                    vpt[:, t, :, 0:DH],
                    ps[:].rearrange("p (h d) -> p h d", h=HPC),
                    dm01[:, t:t + 1],
                )

            def op_unit(j, dt, o_all):
                pp = psA.tile([128, QT], F32, tag="pa", name="pp")
                for kt in range(2):
                    nc.tensor.matmul(
                        pp[:],
                        wp[:, kt, 128 * dt:128 * dt + 128],
                        o_all[:, kt, :],
                        start=(kt == 0), stop=(kt == 1),
                    )
                ob = obp.tile([128, QT], ODT, tag="ob")
                if dt % 2 == 0:
                    nc.vector.tensor_copy(ob[:], pp[:])
                else:
                    nc.scalar.copy(ob[:], pp[:])
                nc.sync.dma_start(
                    out=out_d[128 * dt:128 * dt + 128, QT * j:QT * j + QT],
                    in_=ob[:],
                )

            def need_k(j):
                return (nj[j] + 3) // 4

            # ---- upfront: Q(0), K blocks for tile 0, V k-tiles < nj[0];
            # plus j=1's V tiles and Q(1) to fill the load tail ----
            for m in range(2):
                q_chain(0, m, "scalar")
            for bk in range(need_k(0)):
                for m in range(2):
                    k_chain(bk, m, "scalar")
            for t in range(min(nj[1] if NQT > 1 else nj[0], nj[0] + 2)):
                v_chain(t)
            if NQT > 1:
                for m in range(2):
                    q_chain(1, m, "scalar")

            o_alls = []
            for j in range(NQT):
                # filler units: emitted between attention iterations to
                # keep the (in-order) PE stream dense while ScalarE exps.
                units = []
                if j + 1 < NQT:
                    done_v = (min(nj[1], nj[0] + 2) if j == 0
                              else nj[j])
                    for t in range(max(nj[j], done_v), nj[j + 1]):
                        units.append((v_chain, (t,)))
                    if j > 0:
                        for m in range(2):
                            units.append((q_chain, (j + 1, m,
                                                    "vector" if m else "scalar")))
                    for bk in range(need_k(j), need_k(j + 1)):
                        for m in range(2):
                            units.append((k_chain, (bk, m,
                                                    "vector" if m else "scalar")))
                reserved = []
                if j > 0:
                    for dt in range(D // 128 - 4):
                        units.append((op_unit, (j - 1, dt, o_alls[j - 1])))
                    for dt in range(D // 128 - 4, D // 128):
                        reserved.append((op_unit, (j - 1, dt, o_alls[j - 1])))
                ni = 2 * nj[j]
                emit_at = {}
                for u in range(len(units)):
                    emit_at.setdefault(u * ni // len(units), []).append(units[u])

                o_all = sm.tile([128, 2, QT], MDT, tag="oall")
                o_alls.append(o_all)
                mask_start = ms[j]
                for m in range(2):  # head pairs (2m, 2m+1)
                    o_ps = psO.tile([DH + 1, 2, QT], F32, tag="ops")
                    for i in range(nj[j]):
                        if geom is None:
                            r = i - 4 * j
                            w0 = 128 * r if r > 0 else 0
                        else:
                            r = -1
                            w0 = w0s[j][i]
                        boundary = geom is not None and i >= mask_start
                        ps_s = psS.tile([128, 2, QT], F32, tag="ps")
                        for u in range(2):
                            p0 = 64 * u
                            nc.tensor.matmul(
                                ps_s[:, u, w0:QT],
                                kTn[i // 4][p0:p0 + 64, m,
                                            128 * (i % 4):128 * (i % 4) + 128],
                                qTn[j][p0:p0 + 64, m, w0:QT],
                                start=True, stop=not boundary,
                            )
                        if boundary:
                            # causal/pad mask: accumulate -240 into masked
                            # score elements (exp then yields ~1e-13)
                            for u in range(2):
                                nc.tensor.matmul(
                                    ps_s[:, u, w0:QT],
                                    idm[:],
                                    bm[:, j, i - mask_start, w0:QT],
                                    start=False, stop=True,
                                )
                        wt = wtp.tile([128, 2, QT], MDT, tag="wt")
                        nc.scalar.activation(
                            wt[:, :, w0:QT], ps_s[:, :, w0:QT], EXP,
                            bias=0.0, scale=SCALE)
                        for u in range(2):
                            if geom is None and r >= 0:
                                # causal 128x128 diagonal block
                                nc.vector.tensor_mul(
                                    wt[:, u, w0:w0 + KT],
                                    wt[:, u, w0:w0 + KT],
                                    tri[:],
                                )
                            nc.tensor.matmul(
                                o_ps[:, u, w0:QT],
                                vpt[:, i, 2 * m + u, :],
                                wt[:, u, w0:QT],
                                start=(i == 0), stop=(i == nj[j] - 1),
                            )
                        for fn, args in emit_at.get(m * nj[j] + i, ()):
                            fn(*args)
                    # fast PSUM evacuation, then normalization off the
                    # critical path: r = dm_q / (sums + eps)
                    o_sb = sm.tile([DH + 1, 2, QT], MDT, tag="osb")
                    nc.vector.tensor_copy(o_sb[:], o_ps[:])
                    r0 = sm.tile([1, 2, QT], F32, tag="r0")
                    nc.vector.tensor_scalar_add(
                        r0[:], o_sb[DH:DH + 1, :, :], 1e-30)
                    rf = sm.tile([1, 2, QT], F32, tag="rf")
                    nc.vector.reciprocal_approx_fast(out=rf[:], in_=r0[:])
                    r2 = sm.tile([1, 2, QT], MDT, tag="r2")
                    nc.vector.tensor_mul(r2[:], rf[:], dmr[:, j, :, :])
                    rb = sm.tile([64, 2, QT], MDT, tag="rb")
                    nc.gpsimd.partition_broadcast(rb[:], r2[:], channels=64)
                    for u in range(2):
                        nc.vector.tensor_mul(
                            o_all[64 * u:64 * u + 64, m, :],
                            o_sb[0:DH, u, :], rb[:, u, :],
                        )
                for fn, args in reserved:
                    fn(*args)

            # ---- output projection for the last q tile ----
            for dt in range(D // 128):
                op_unit(NQT - 1, dt, o_alls[NQT - 1])

    nc.finalize()
    return nc


def _geometry(data_mask):
    """Compacted-key geometry shared by all cores, or None if the mask
    cannot compact profitably (then the dense program is used).

    Returns (TC, nj, ms, GBD): compacted+padded key count, number of
    k tiles per q tile, first boundary-masked tile per q tile, and the
    boundary-mask window depth."""
    dm = np.asarray(data_mask) != 0
    valids = [np.where(dm[b])[0] for b in range(B)]
    cnt = max(len(v) for v in valids)
    if cnt == 0:
        return None
    TC = KT * int(np.ceil(cnt / KT))
    if TC >= T:
        return None
    nj, ms, w0s = [], [], []
    for j in range(NQT):
        c = max(int(np.searchsorted(v, QT * (j + 1))) for v in valids)
        nj.append(max(int(np.ceil(c / KT)), 1))
        # first tile (over any batch) containing a key beyond q-tile j's
        # first position: earlier tiles are fully valid for every q
        m = min(int(np.searchsorted(v, QT * j + 1)) // KT for v in valids)
        ms.append(min(m, nj[j]))
        # per-tile column trim: every key in tile i sits at pos >=
        # min_b pos_b[128i], so q columns below that are fully masked
        w = []
        for i in range(nj[j]):
            pmin = min((int(v[KT * i]) if KT * i < len(v) else 1 << 30)
                       for v in valids)
            w.append(int(np.clip(pmin - QT * j, 0, QT - 1)))
        w0s.append(tuple(w))
    gbd = max(max(nj[j] - ms[j] for j in range(NQT)), 1)
    if gbd > 6:
        return None
    return TC, tuple(nj), tuple(ms), gbd, tuple(w0s)


def _make_in_maps(x, data_mask, Wq, Wk, Wv, Wp, mm_dtype=None, geom=None):
    if (mm_dtype or MM_DTYPE) == "bfloat16":
        import ml_dtypes
        mdt = ml_dtypes.bfloat16
    else:
        mdt = np.float32
    x = np.ascontiguousarray(np.asarray(x, np.float32))
    dm = np.asarray(data_mask).astype(np.float32)

    def wsplit(W, sl, colslice):
        W = np.asarray(W, np.float32)
        Wc = W[:, sl] if colslice else W[sl, :]
        nb = Wc.shape[0] // 128
        return np.ascontiguousarray(
            Wc.reshape(nb, 128, Wc.shape[1]).transpose(1, 0, 2).astype(mdt))

    p = np.arange(KT)[:, None]
    q = np.arange(KT)[None, :]
    tri = (q >= p).astype(np.float32).astype(mdt)

    in_maps = []
    for c in range(NC):
        b, g = divmod(c, HPC)
        sl = slice(DC * g, DC * g + DC)
        dmb = dm[b]
        dmr = np.repeat(dmb.reshape(NQT, 1, QT), 2, axis=1)[None]
        im = {
            "xT": np.ascontiguousarray(x[b].T.astype(mdt)),
            "wq": wsplit(Wq, sl, True),
            "wk": wsplit(Wk, sl, True),
            "wv": wsplit(Wv, sl, True),
            "wp": wsplit(Wp, sl, False),
            "dmrow2": np.ascontiguousarray(dmr),
        }
        if geom is None:
            im["dm01"] = np.ascontiguousarray(dmb.reshape(NKT, KT).T)
            im["tri"] = tri
        else:
            TC, nj, ms, GBD, _w0s = geom
            valid = np.where(dmb != 0)[0]
            cnt = len(valid)
            xk = np.zeros((D, TC), np.float32)
            xk[:, :cnt] = x[b].T[:, valid]
            im["xk"] = np.ascontiguousarray(xk.astype(mdt))
            dm01c = np.zeros((TC // KT, KT), np.float32)
            dm01c.reshape(-1)[:cnt] = 1.0
            im["dm01"] = np.ascontiguousarray(dm01c.T)
            pos = np.full(TC, 1 << 30, np.int64)
            pos[:cnt] = valid
            bmv = np.zeros((KT, NQT, GBD, QT), np.float32)
            for j in range(NQT):
                for gg in range(GBD):
                    i = ms[j] + gg
                    if i >= nj[j]:
                        break
                    kp = pos[KT * i:KT * i + KT]  # [128]
                    qq = QT * j + np.arange(QT)   # [512]
                    bmv[:, j, gg, :] = np.where(
                        kp[:, None] <= qq[None, :], 0.0, -240.0)
            im["bm"] = np.ascontiguousarray(bmv.astype(mdt))
            im["idm"] = np.ascontiguousarray(np.eye(KT, dtype=np.float32).astype(mdt))
        in_maps.append(im)
    return in_maps


def _postprocess(results, data_mask, bp):
    out = np.empty((B, T, D), np.float32)
    for b in range(B):
        acc = results[HPC * b]["outT"].astype(np.float32)
        for g in range(1, HPC):
            acc = acc + results[HPC * b + g]["outT"].astype(np.float32)
        out[b] = acc.T
    bp = np.asarray(bp, np.float32)
    if np.any(bp):
        # general path: device skipped bp and the final row mask folding
        # assumes bp == 0, so apply both here
        out = (out + bp) * np.asarray(data_mask, np.float32)[..., None]
    return out


def _numpy_reference(x, data_mask, Wq, bq, Wk, bk, Wv, bv, Wp, bp):
    # general fallback (only used when q/k/v biases are nonzero, which
    # does not happen for this problem's setup_inputs)
    x = np.asarray(x, np.float64)
    dm = np.asarray(data_mask) != 0
    q = (x @ np.asarray(Wq, np.float64) + np.asarray(bq, np.float64))
    k = (x @ np.asarray(Wk, np.float64) + np.asarray(bk, np.float64))
    v = (x @ np.asarray(Wv, np.float64) + np.asarray(bv, np.float64))
    q = q.reshape(B, T, H, DH).transpose(0, 2, 1, 3) * SCALE
    k = k.reshape(B, T, H, DH).transpose(0, 2, 1, 3)
    v = v.reshape(B, T, H, DH).transpose(0, 2, 1, 3)
    causal = np.tril(np.ones((T, T), bool))
    out = np.empty((B, T, D), np.float64)
    for b in range(B):
        mask = causal & dm[b][:, None] & dm[b][None, :]
        for h in range(H):
            s = q[b, h] @ k[b, h].T
            s = np.where(mask, s, -np.inf)
            s -= np.max(s, axis=-1, keepdims=True)
            w = np.exp(s)
            denom = w.sum(-1, keepdims=True)
            w = np.where(denom > 0, w / np.where(denom == 0, 1, denom), 0.0)
            w = np.nan_to_num(w)
            out[b, :, h * DH:(h + 1) * DH] = w @ v[b, h]
    out = out @ np.asarray(Wp, np.float64) + np.asarray(bp, np.float64)
    out *= dm[..., None]
    return out.astype(np.float32)


def kernel(x, data_mask, Wq, bq, Wk, bk, Wv, bv, Wp, bp):
    if any(np.any(np.asarray(v)) for v in (bq, bk, bv)):
        return _numpy_reference(x, data_mask, Wq, bq, Wk, bk, Wv, bv, Wp, bp)

    from concourse.bass_utils import run_bass_kernel_spmd

    geom = _geometry(data_mask)
    key = ("nc", geom)
    if key not in _cached:
        _cached[key] = _build_program(geom=geom)
    nc = _cached[key]
    in_maps = _make_in_maps(x, data_mask, Wq, Wk, Wv, Wp, geom=geom)
    res = run_bass_kernel_spmd(nc, in_maps, core_ids=list(range(NC)))
    return _postprocess(res.results, data_mask, bp)
